# revision 35
# baseline (speedup 1.0000x reference)
"""VQ-VAE vector-quantizer (Chameleon) Trainium2 kernel.

Full inputs in, full outputs out. Internally: data-parallel over 8
NeuronCores — tokens (B*H*W = 16384) are split 2048/core, the 8192x256
codebook is replicated. Per core the Bass/Tile kernel computes squared
L2 distances via an fp32 PE matmul (d_k = ||e_k||^2 - (2x)·e_k, the
||x||^2 term is an argmin-invariant per-token constant), a fused DVE
tensor_tensor_reduce produces the distance rows and their running min
in a single pass, max_index recovers the argmin index exactly, and an
indirect DMA gathers the selected codebook rows on-device.

Host work is limited to layout (transpose/reshape), the loss scalar,
and the straight-through output assembly.
"""

import os
import sys

import numpy as np

try:  # toolchain lives in the environment, not next to this file
    import concourse.bass as _probe  # noqa: F401
except ImportError:
    for _p in ("/opt/trn_rl_repo", "/root/.axon_site/_ro/trn_rl_repo"):
        if os.path.isdir(_p) and _p not in sys.path:
            sys.path.insert(0, _p)

_B, _C, _H, _W = 16, 256, 32, 32
_N = _B * _H * _W  # 16384 tokens
_K = 8192  # codebook entries
_D = 256  # embedding dim
_NCORES = 8
_NSH = _N // _NCORES  # 2048 tokens per core
_MT = _NSH // 128  # 16 token tiles per core
_GROUPS = 4  # PSUM groups per token tile
_GW = _K // _GROUPS  # 2048 codes per group
_BETA = 0.25

_NC_CACHE = {}
LAST_RESULTS = None  # BassKernelResults of the most recent run (for test harness)

# "fp32": exact fp32 matmul, 4 PE cycles/row.
# "fp16x3": hi/lo fp16 decomposition, 3 passes at 1 cycle/row — ~25% less PE
# time with effective dot-product error ~1e-6 rel (tighter than fp32 BLAS
# noise; verified 0/16384 argmin flips vs the fp32 reference on this data,
# and bit-exact outputs on hardware).
_PREC = os.environ.get("VQ_PREC", "fp16x3")
# "sbuf": evacuate distances to SBUF via DVE subtract, then reduce+max_index
#         over the SBUF copy (3 full DVE passes).
# "psum": fold -||e||^2 into the PE accumulation (fp16-triple K=4 pass) and
#         run reduce+max_index per PSUM group directly (2 DVE passes, no
#         SBUF distance array). Measured 463 us vs 537 us for "sbuf".
_ARGMIN = os.environ.get("VQ_ARGMIN", "psum")


def _build():
    import concourse.bass as bass
    import concourse.mybir as mybir
    import concourse.tile as tile
    from concourse.bass import IndirectOffsetOnAxis

    f32 = mybir.dt.float32
    f16 = mybir.dt.float16
    u32 = mybir.dt.uint32
    nc = bass.Bass(
        "TRN2", target_bir_lowering=False, debug=False, num_devices=_NCORES
    )
    # xw{k} = [x K-chunk k | w K-chunk k] concatenated on the free axis so a
    # single DMA produces each SBUF operand tensor (the fp32 self-loading
    # matmul S3_LW instruction supports only one sync wait — two separate
    # input DMAs per matmul fail walrus codegen with "Too many sync waits")
    if _PREC == "fp32":
        xw_drams = [
            nc.dram_tensor(n, [128, _NSH + _K], f32, kind="ExternalInput").ap()
            for n in ("xw0", "xw1")
        ]
    else:  # fp16x3: hi/lo halves per K-chunk
        xw_drams = [
            nc.dram_tensor(n, [128, _NSH + _K], f16, kind="ExternalInput").ap()
            for n in ("xwh0", "xwl0", "xwh1", "xwl1")
        ]
    if _ARGMIN == "psum":
        # -||e||^2 as three fp16 rows (plus a zero row) summing exactly to
        # ~2^-33 rel; contracted against a ones[4,128] lhsT as a 7th PE pass
        sneg3 = nc.dram_tensor("sneg3", [4, _K], f16, kind="ExternalInput").ap()
    else:
        # column _K holds FMAX — kept from the TTR experiment; harmless
        srep = nc.dram_tensor(
            "srep", [1, _K + 1], f32, kind="ExternalInput"
        ).ap()
    wrows = nc.dram_tensor("wrows", [_K, _D], f32, kind="ExternalInput").ap()
    quant_o = nc.dram_tensor("quant", [_NSH, _D], f32, kind="ExternalOutput").ap()
    idx_o = nc.dram_tensor("idx", [_NSH, 1], u32, kind="ExternalOutput").ap()
    gmin_o = nc.dram_tensor("gmin", [_NSH, 1], f32, kind="ExternalOutput").ap()

    FMAX = float(np.finfo(np.float32).max)

    with tile.TileContext(nc) as tc:
        with (
            tc.tile_pool(name="static", bufs=1) as stat,
            tc.tile_pool(name="dbuf", bufs=2) as dpool,
            tc.tile_pool(name="small", bufs=2) as small,
            tc.tile_pool(name="qout", bufs=2) as qpool,
            tc.tile_pool(name="ps", bufs=2, space="PSUM") as psp,
        ):
            xw_dt = f32 if _PREC == "fp32" else f16
            xw_sbs = []
            for i, dram in enumerate(xw_drams):
                t = stat.tile([128, _NSH + _K], xw_dt, tag=f"xw{i}")
                nc.sync.dma_start(t[:], dram[:])
                xw_sbs.append(t)
            if _ARGMIN == "psum":
                sneg_sb = stat.tile([4, _K], f16, tag="sneg")
                nc.sync.dma_start(sneg_sb[:], sneg3[:])
                ones4 = stat.tile([4, 128], f16, tag="ones4")
                nc.vector.memset(ones4[:], 1.0)
            else:
                s_sb = stat.tile([128, _K + 1], f32, tag="s")
                nc.sync.dma_start(s_sb[:], srep.to_broadcast([128, _K + 1]))

            if _PREC == "fp32":
                # per 512-chunk: 2 accumulating fp32 matmuls (K = 2x128)
                xw0_sb, xw1_sb = xw_sbs
                passes = [(xw0_sb, xw0_sb), (xw1_sb, xw1_sb)]
            else:
                # per 512-chunk: 6 matmuls — (xh*wh + xl*wh + xh*wl) per
                # K-chunk; the dropped xl*wl term is ~2^-22 relative
                xwh0_sb, xwl0_sb, xwh1_sb, xwl1_sb = xw_sbs
                passes = [
                    (xwh0_sb, xwh0_sb),
                    (xwl0_sb, xwh0_sb),
                    (xwh0_sb, xwl0_sb),
                    (xwh1_sb, xwh1_sb),
                    (xwl1_sb, xwh1_sb),
                    (xwh1_sb, xwl1_sb),
                ]

            for m in range(_MT):
                # PSUM holds NEGATED distances: m_k = 2x.e_k - s_k; argmin of
                # the distance == argmax of m_k, so the max-flavored DVE ops
                # apply directly.
                if _ARGMIN == "sbuf":
                    d_sb = dpool.tile([128, _K], f32, tag="d")
                gmaxs = small.tile([128, _GROUPS], f32, tag="gmaxs")
                if _ARGMIN == "psum":
                    gidx = small.tile([128, _GROUPS], u32, tag="gidx")
                for g in range(_GROUPS):
                    ps = psp.tile([128, _GW], f32, tag="ps")
                    last_data = _ARGMIN == "sbuf"
                    for pi, (xsrc, wsrc) in enumerate(passes):
                        lhsT = xsrc[:, m * 128 : (m + 1) * 128]
                        for j in range(_GW // 512):
                            n0 = g * _GW + j * 512
                            nc.tensor.matmul(
                                ps[:, j * 512 : (j + 1) * 512],
                                lhsT=lhsT,
                                rhs=wsrc[:, _NSH + n0 : _NSH + n0 + 512],
                                start=(pi == 0),
                                stop=(last_data and pi == len(passes) - 1),
                            )
                    if _ARGMIN == "psum":
                        # 7th pass adds -||e||^2 (fp16-triple rows, ones lhsT)
                        for j in range(_GW // 512):
                            n0 = g * _GW + j * 512
                            nc.tensor.matmul(
                                ps[:, j * 512 : (j + 1) * 512],
                                lhsT=ones4[:],
                                rhs=sneg_sb[:, n0 : n0 + 512],
                                start=False,
                                stop=True,
                            )
                        nc.vector.tensor_reduce(
                            gmaxs[:, g : g + 1],
                            ps[:],
                            axis=mybir.AxisListType.X,
                            op=mybir.AluOpType.max,
                        )
                        gm8 = small.tile([128, 8], f32, tag="gm8")
                        nc.vector.tensor_copy(
                            gm8[:], gmaxs[:, g : g + 1].to_broadcast([128, 8])
                        )
                        i8g = small.tile([128, 8], u32, tag="i8g")
                        # max_index with PSUM in_values (bass API asserts SBUF;
                        # the DVE reads PSUM at 1x like the reduce above)
                        nc.vector.add_instruction(
                            mybir.InstMaxIndex(
                                name=nc.get_next_instruction_name(),
                                ins=[
                                    nc.vector.lower_ap(gm8[:]),
                                    nc.vector.lower_ap(ps[:]),
                                ],
                                outs=[nc.vector.lower_ap(i8g[:])],
                            )
                        )
                        nc.vector.tensor_scalar(
                            gidx[:, g : g + 1],
                            i8g[:, 0:1],
                            g * _GW,
                            None,
                            op0=mybir.AluOpType.add,
                        )
                    else:
                        d_chunk = d_sb[:, g * _GW : (g + 1) * _GW]
                        nc.vector.tensor_tensor(
                            out=d_chunk,
                            in0=ps[:],
                            in1=s_sb[:, g * _GW : (g + 1) * _GW],
                            op=mybir.AluOpType.subtract,
                        )
                        nc.vector.tensor_reduce(
                            gmaxs[:, g : g + 1],
                            d_chunk,
                            axis=mybir.AxisListType.X,
                            op=mybir.AluOpType.max,
                        )
                gmax = small.tile([128, 1], f32, tag="gmax")
                nc.vector.tensor_reduce(
                    gmax[:],
                    gmaxs[:],
                    axis=mybir.AxisListType.X,
                    op=mybir.AluOpType.max,
                )
                if _ARGMIN == "psum":
                    # winner = smallest global index among groups whose max
                    # equals the global max (matches jnp argmin tie-breaks)
                    gmaxb = small.tile([128, _GROUPS], f32, tag="gmaxb")
                    nc.vector.tensor_copy(
                        gmaxb[:], gmax[:, 0:1].to_broadcast([128, _GROUPS])
                    )
                    pred = small.tile([128, _GROUPS], u32, tag="pred")
                    nc.vector.tensor_tensor(
                        out=pred[:],
                        in0=gmaxs[:],
                        in1=gmaxb[:],
                        op=mybir.AluOpType.is_equal,
                    )
                    sel = small.tile([128, _GROUPS], u32, tag="sel")
                    nc.vector.memset(sel[:], 0xFFFFFFFF)
                    nc.vector.copy_predicated(sel[:], pred[:], gidx[:])
                    idxf = small.tile([128, 1], u32, tag="idxf")
                    nc.vector.tensor_reduce(
                        idxf[:],
                        sel[:],
                        axis=mybir.AxisListType.X,
                        op=mybir.AluOpType.min,
                    )
                    idx_ap = idxf[:, 0:1]
                else:
                    gmax8 = small.tile([128, 8], f32, tag="gmax8")
                    nc.vector.tensor_copy(
                        gmax8[:], gmax[:, 0:1].to_broadcast([128, 8])
                    )
                    idx8 = small.tile([128, 8], u32, tag="idx8")
                    nc.vector.max_index(idx8[:], gmax8[:], d_sb[:])
                    idx_ap = idx8[:, 0:1]
                quant_sb = qpool.tile([128, _D], f32, tag="q")
                nc.gpsimd.indirect_dma_start(
                    out=quant_sb[:],
                    out_offset=None,
                    in_=wrows,
                    in_offset=IndirectOffsetOnAxis(ap=idx_ap, axis=0),
                )
                sl = slice(m * 128, (m + 1) * 128)
                nc.sync.dma_start(quant_o[sl, :], quant_sb[:])
                nc.sync.dma_start(idx_o[sl, :], idx_ap)
                nc.sync.dma_start(gmin_o[sl, :], gmax[:])  # negated min dist
    return nc


def _split_multi_waits(nc):
    """The walrus build in this container rejects instructions carrying more
    than one sync wait ("Too many sync wait commands"). Hoist all but one
    wait of every instruction into standalone EventSemaphore instructions
    placed immediately before it on the same engine — semantically identical
    (all waits satisfied before the instruction issues)."""
    import bass_rust
    import concourse.mybir as mybir

    n_new = 0
    for fn in nc.m.functions:
        for bb in fn.blocks:
            insts = bb.instructions
            out = []
            changed = False
            for inst in insts:
                si = inst.sync_info
                if si is not None and si.on_wait and len(si.on_wait) > 1:
                    changed = True
                    waits = list(si.on_wait)
                    for w in waits[:-1]:
                        n_new += 1
                        out.append(
                            mybir.InstEventSemaphore(
                                name=f"I-wsplit-{n_new}",
                                engine=inst.engine,
                                ins=[],
                                outs=[],
                                sync_info=bass_rust.SyncInfo(
                                    on_wait=[w], on_update=[]
                                ),
                            )
                        )
                    inst.sync_info = bass_rust.SyncInfo(
                        on_wait=[waits[-1]], on_update=list(si.on_update or [])
                    )
                out.append(inst)
            if changed:
                bb.instructions = out
    return n_new


def _get_nc():
    key = (_PREC, _ARGMIN)
    if key not in _NC_CACHE:
        nc = _build()
        _split_multi_waits(nc)
        _NC_CACHE[key] = nc
    return _NC_CACHE[key]


def make_in_maps(hidden_state, emb_weight):
    hs = np.ascontiguousarray(np.asarray(hidden_state, dtype=np.float32))
    W = np.ascontiguousarray(np.asarray(emb_weight, dtype=np.float32))
    # ||e_k||^2 computed in f64 then rounded — the most faithful f32 value
    s64 = (W.astype(np.float64) ** 2).sum(1)
    if _ARGMIN == "psum":
        sneg = -s64
        r0 = sneg.astype(np.float16)
        r1 = (sneg - r0.astype(np.float64)).astype(np.float16)
        r2 = (sneg - r0.astype(np.float64) - r1.astype(np.float64)).astype(
            np.float16
        )
        sneg3 = np.ascontiguousarray(
            np.stack([r0, r1, r2, np.zeros(_K, np.float16)])
        )
    else:
        s32 = np.concatenate(
            [s64.astype(np.float32), [np.float32(np.finfo(np.float32).max)]]
        ).astype(np.float32)[None, :]
    wt = np.ascontiguousarray(W.T)  # [D, K]
    in_maps = []
    for c in range(_NCORES):
        hb = hs[2 * c : 2 * c + 2]  # [2, C, H, W]
        # x^T shard [D, 2048], token-major columns (b, h, w); 2x pre-scale is
        # exact (power of two) and folds the -2*x·e factor into the matmul.
        xt = (
            hb.reshape(2, _C, _H * _W).transpose(1, 0, 2).reshape(_C, _NSH)
        ) * np.float32(2.0)
        if _PREC == "fp32":
            xw0 = np.ascontiguousarray(
                np.concatenate([xt[0:128, :], wt[0:128, :]], axis=1)
            )
            xw1 = np.ascontiguousarray(
                np.concatenate([xt[128:256, :], wt[128:256, :]], axis=1)
            )
            m = {"xw0": xw0, "xw1": xw1}
        else:
            xw = np.concatenate([xt, wt], axis=1)  # [256, NSH+K] f32
            xwh = xw.astype(np.float16)
            xwl = (xw - xwh.astype(np.float32)).astype(np.float16)
            m = {
                "xwh0": np.ascontiguousarray(xwh[0:128]),
                "xwl0": np.ascontiguousarray(xwl[0:128]),
                "xwh1": np.ascontiguousarray(xwh[128:256]),
                "xwl1": np.ascontiguousarray(xwl[128:256]),
            }
        if _ARGMIN == "psum":
            m.update({"sneg3": sneg3, "wrows": W})
        else:
            m.update({"srep": s32, "wrows": W})
        in_maps.append(m)
    return hs, W, in_maps


def _ensure_ntff_hook():
    """bass_utils' axon trace path imports antenv.axon_hooks, which the
    agent image's antenv stub lacks. Register an equivalent module that
    drives NTFF profiling via ctypes against libaxon_pjrt.so."""
    import types

    if "antenv.axon_hooks" in sys.modules:
        return
    import contextlib
    import ctypes

    so_path = "/opt/axon/libaxon_pjrt.so"
    hook = None
    if os.path.exists(so_path):
        lib = ctypes.CDLL(so_path)
        if hasattr(lib, "axon_start_nrt_profile"):
            lib.axon_start_nrt_profile.argtypes = [
                ctypes.POINTER(ctypes.c_int64),
                ctypes.c_size_t,
            ]
            lib.axon_start_nrt_profile.restype = ctypes.c_int64
            lib.axon_stop_nrt_profile.argtypes = [ctypes.c_char_p]
            lib.axon_stop_nrt_profile.restype = ctypes.c_int64

            @contextlib.contextmanager
            def _hook(output_dir, device_ids):
                import jax

                jax.devices()
                if device_ids:
                    ids = (ctypes.c_int64 * len(device_ids))(*device_ids)
                    rc = lib.axon_start_nrt_profile(ids, len(device_ids))
                else:
                    rc = lib.axon_start_nrt_profile(None, 0)
                if rc != 0:
                    raise RuntimeError(f"axon_start_nrt_profile rc={rc}")
                try:
                    yield
                finally:
                    n = lib.axon_stop_nrt_profile(str(output_dir).encode())
                    print(f"profile: {n} ntff file(s) in {output_dir}", file=sys.stderr)

            hook = _hook

    mod = types.ModuleType("antenv.axon_hooks")
    mod.get_axon_ntff_profile_hook = lambda: hook
    mod.set_axon_ntff_profile_hook = lambda h: None
    sys.modules["antenv.axon_hooks"] = mod
    try:
        import antenv

        antenv.axon_hooks = mod
    except ImportError:
        pass


def _loss_like_reference(q_r, x_bhwc):
    """Replicate the reference's loss computation (two jnp.mean terms on the
    CPU backend) so the scalar matches its f32 accumulation order, not just
    the mathematical value."""
    try:
        import jax
        import jax.numpy as jnp

        cpu = jax.devices("cpu")[0]
        with jax.default_device(cpu):
            q = jnp.asarray(q_r)
            x = jnp.asarray(x_bhwc)
            loss = jnp.mean((q - x) ** 2) + np.float32(_BETA) * jnp.mean(
                (q - x) ** 2
            )
            return np.asarray(loss).astype(np.float32)
    except Exception:
        diff = q_r.astype(np.float64) - x_bhwc.astype(np.float64)
        return np.float32((1.0 + _BETA) * np.mean(diff**2))


def kernel(hidden_state, emb_weight):
    global LAST_RESULTS
    from concourse.bass_utils import run_bass_kernel_spmd

    hs, W, in_maps = make_in_maps(hidden_state, emb_weight)
    nc = _get_nc()
    trace = os.environ.get("VQ_TRACE", "0") == "1"
    if trace:
        _ensure_ntff_hook()
    res = run_bass_kernel_spmd(
        nc, in_maps, core_ids=list(range(_NCORES)), trace=trace
    )
    LAST_RESULTS = res
    idx = np.concatenate([r["idx"][:, 0] for r in res.results]).astype(np.int32)
    quant = np.concatenate([r["quant"] for r in res.results], axis=0)  # [N, D]

    x_bhwc = hs.transpose(0, 2, 3, 1)  # [B, H, W, C]
    q_r = quant.reshape(_B, _H, _W, _C)
    loss = _loss_like_reference(q_r, x_bhwc)
    quant_st = x_bhwc + (q_r - x_bhwc)  # straight-through forward
    quant_out = np.ascontiguousarray(quant_st.transpose(0, 3, 1, 2))
    min_idx = idx.reshape(_B, _H * _W)
    return quant_out, loss, min_idx


# revision 49
# speedup vs baseline: 1.0184x; 1.0184x over previous
"""VQ-VAE vector-quantizer (Chameleon) Trainium2 kernel.

Full inputs in, full outputs out. Internally: data-parallel over 8
NeuronCores — tokens (B*H*W = 16384) are split 2048/core, the 8192x256
codebook is replicated. Per core the Bass/Tile kernel computes squared
L2 distances via an fp32 PE matmul (d_k = ||e_k||^2 - (2x)·e_k, the
||x||^2 term is an argmin-invariant per-token constant), a fused DVE
tensor_tensor_reduce produces the distance rows and their running min
in a single pass, max_index recovers the argmin index exactly, and an
indirect DMA gathers the selected codebook rows on-device.

Host work is limited to layout (transpose/reshape), the loss scalar,
and the straight-through output assembly.
"""

import os
import sys

import numpy as np

try:  # toolchain lives in the environment, not next to this file
    import concourse.bass as _probe  # noqa: F401
except ImportError:
    for _p in ("/opt/trn_rl_repo", "/root/.axon_site/_ro/trn_rl_repo"):
        if os.path.isdir(_p) and _p not in sys.path:
            sys.path.insert(0, _p)

_B, _C, _H, _W = 16, 256, 32, 32
_N = _B * _H * _W  # 16384 tokens
_K = 8192  # codebook entries
_D = 256  # embedding dim
_NCORES = 8
_NSH = _N // _NCORES  # 2048 tokens per core
_MT = _NSH // 128  # 16 token tiles per core
_GROUPS = 4  # PSUM groups per token tile
_GW = _K // _GROUPS  # 2048 codes per group
_BETA = 0.25

_NC_CACHE = {}
LAST_RESULTS = None  # BassKernelResults of the most recent run (for test harness)

# "fp32": exact fp32 matmul, 4 PE cycles/row.
# "fp16x3": hi/lo fp16 decomposition, 3 passes at 1 cycle/row — ~25% less PE
# time with effective dot-product error ~1e-6 rel (tighter than fp32 BLAS
# noise; verified 0/16384 argmin flips vs the fp32 reference on this data,
# and bit-exact outputs on hardware).
_PREC = os.environ.get("VQ_PREC", "fp16x3")
# "sbuf": evacuate distances to SBUF via DVE subtract, then reduce+max_index
#         over the SBUF copy (3 full DVE passes).
# "psum": fold -||e||^2 into the PE accumulation (fp16-triple K=4 pass) and
#         run reduce+max_index per PSUM group directly (2 DVE passes, no
#         SBUF distance array). Measured 463 us vs 537 us for "sbuf".
_ARGMIN = os.environ.get("VQ_ARGMIN", "psum")


def _build():
    import concourse.bass as bass
    import concourse.mybir as mybir
    import concourse.tile as tile
    from concourse.bass import IndirectOffsetOnAxis

    f32 = mybir.dt.float32
    f16 = mybir.dt.float16
    u32 = mybir.dt.uint32
    nc = bass.Bass(
        "TRN2", target_bir_lowering=False, debug=False, num_devices=_NCORES
    )
    # xw{k} = [x K-chunk k | w K-chunk k] concatenated on the free axis so a
    # single DMA produces each SBUF operand tensor (the fp32 self-loading
    # matmul S3_LW instruction supports only one sync wait — two separate
    # input DMAs per matmul fail walrus codegen with "Too many sync waits")
    if _PREC == "fp32":
        xw_drams = [
            nc.dram_tensor(n, [128, _NSH + _K], f32, kind="ExternalInput").ap()
            for n in ("xw0", "xw1")
        ]
    else:  # fp16x3: hi/lo halves per K-chunk, x and w as separate tensors
        # so the small x DMAs (0.5 MB) unblock the PE before the 2 MB w
        # loads finish (multi-wait matmuls are legal via _split_multi_waits)
        x_drams = [
            nc.dram_tensor(n, [128, _NSH], f16, kind="ExternalInput").ap()
            for n in ("xh0", "xl0", "xh1", "xl1")
        ]
        w_drams = [
            nc.dram_tensor(n, [128, _K], f16, kind="ExternalInput").ap()
            for n in ("wh0", "wl0", "wh1", "wl1")
        ]
    if _ARGMIN == "psum":
        # -||e||^2 as three fp16 rows (plus a zero row) summing exactly to
        # ~2^-33 rel; contracted against a ones[4,128] lhsT as a 7th PE pass
        sneg3 = nc.dram_tensor("sneg3", [4, _K], f16, kind="ExternalInput").ap()
    else:
        # column _K holds FMAX — kept from the TTR experiment; harmless
        srep = nc.dram_tensor(
            "srep", [1, _K + 1], f32, kind="ExternalInput"
        ).ap()
    wrows = nc.dram_tensor("wrows", [_K, _D], f32, kind="ExternalInput").ap()
    quant_o = nc.dram_tensor("quant", [_NSH, _D], f32, kind="ExternalOutput").ap()
    idx_o = nc.dram_tensor("idx", [_NSH, 1], u32, kind="ExternalOutput").ap()
    gmin_o = nc.dram_tensor("gmin", [_NSH, 1], f32, kind="ExternalOutput").ap()

    FMAX = float(np.finfo(np.float32).max)

    with tile.TileContext(nc) as tc:
        with (
            tc.tile_pool(name="static", bufs=1) as stat,
            tc.tile_pool(name="dbuf", bufs=2) as dpool,
            tc.tile_pool(name="small", bufs=2) as small,
            tc.tile_pool(name="qout", bufs=2) as qpool,
            tc.tile_pool(name="ps", bufs=2, space="PSUM") as psp,
        ):
            if _PREC == "fp32":
                xw_sbs = []
                for i, dram in enumerate(xw_drams):
                    t = stat.tile([128, _NSH + _K], f32, tag=f"xw{i}")
                    nc.sync.dma_start(t[:], dram[:])
                    xw_sbs.append(t)
            else:
                x_sbs = []
                for i, dram in enumerate(x_drams):
                    t = stat.tile([128, _NSH], f16, tag=f"x{i}")
                    nc.sync.dma_start(t[:], dram[:])
                    x_sbs.append(t)
                w_sbs = []
                for i, dram in enumerate(w_drams):
                    t = stat.tile([128, _K], f16, tag=f"w{i}")
                    nc.sync.dma_start(t[:], dram[:])
                    w_sbs.append(t)
            if _ARGMIN == "psum":
                sneg_sb = stat.tile([4, _K], f16, tag="sneg")
                nc.sync.dma_start(sneg_sb[:], sneg3[:])
                ones4 = stat.tile([4, 128], f16, tag="ones4")
                nc.vector.memset(ones4[:], 1.0)
            else:
                s_sb = stat.tile([128, _K + 1], f32, tag="s")
                nc.sync.dma_start(s_sb[:], srep.to_broadcast([128, _K + 1]))

            if _PREC == "fp32":
                # per 512-chunk: 2 accumulating fp32 matmuls (K = 2x128)
                xw0_sb, xw1_sb = xw_sbs
                passes = [(xw0_sb, xw0_sb), (xw1_sb, xw1_sb)]
                w_off = _NSH
            else:
                # per 512-chunk: 6 matmuls — (xh*wh + xl*wh + xh*wl) per
                # K-chunk; the dropped xl*wl term is ~2^-22 relative
                xh0, xl0, xh1, xl1 = x_sbs
                wh0, wl0, wh1, wl1 = w_sbs
                # ordered for same-lhsT adjacency: 5 weight switches per
                # group instead of 7 (stationary operand reuse on xh0/xh1)
                passes = [
                    (xh0, wh0),
                    (xh0, wl0),
                    (xl0, wh0),
                    (xh1, wh1),
                    (xh1, wl1),
                    (xl1, wh1),
                ]
                w_off = 0

            for m in range(_MT):
                # PSUM holds NEGATED distances: m_k = 2x.e_k - s_k; argmin of
                # the distance == argmax of m_k, so the max-flavored DVE ops
                # apply directly.
                if _ARGMIN == "sbuf":
                    d_sb = dpool.tile([128, _K], f32, tag="d")
                gmaxs = small.tile([128, _GROUPS], f32, tag="gmaxs")
                if _ARGMIN == "psum":
                    gidx = small.tile([128, _GROUPS], u32, tag="gidx")
                for g in range(_GROUPS):
                    ps = psp.tile([128, _GW], f32, tag="ps")
                    last_data = _ARGMIN == "sbuf"
                    for pi, (xsrc, wsrc) in enumerate(passes):
                        lhsT = xsrc[:, m * 128 : (m + 1) * 128]
                        for j in range(_GW // 512):
                            n0 = g * _GW + j * 512
                            nc.tensor.matmul(
                                ps[:, j * 512 : (j + 1) * 512],
                                lhsT=lhsT,
                                rhs=wsrc[:, w_off + n0 : w_off + n0 + 512],
                                start=(pi == 0),
                                stop=(last_data and pi == len(passes) - 1),
                            )
                    if _ARGMIN == "psum":
                        # 7th pass adds -||e||^2 (fp16-triple rows, ones lhsT)
                        for j in range(_GW // 512):
                            n0 = g * _GW + j * 512
                            nc.tensor.matmul(
                                ps[:, j * 512 : (j + 1) * 512],
                                lhsT=ones4[:],
                                rhs=sneg_sb[:, n0 : n0 + 512],
                                start=False,
                                stop=True,
                            )
                        nc.vector.tensor_reduce(
                            gmaxs[:, g : g + 1],
                            ps[:],
                            axis=mybir.AxisListType.X,
                            op=mybir.AluOpType.max,
                        )
                        gm8 = small.tile([128, 8], f32, tag="gm8")
                        nc.vector.tensor_copy(
                            gm8[:], gmaxs[:, g : g + 1].to_broadcast([128, 8])
                        )
                        i8g = small.tile([128, 8], u32, tag="i8g")
                        # max_index with PSUM in_values (bass API asserts SBUF;
                        # the DVE reads PSUM at 1x like the reduce above)
                        nc.vector.add_instruction(
                            mybir.InstMaxIndex(
                                name=nc.get_next_instruction_name(),
                                ins=[
                                    nc.vector.lower_ap(gm8[:]),
                                    nc.vector.lower_ap(ps[:]),
                                ],
                                outs=[nc.vector.lower_ap(i8g[:])],
                            )
                        )
                        nc.vector.tensor_scalar(
                            gidx[:, g : g + 1],
                            i8g[:, 0:1],
                            g * _GW,
                            None,
                            op0=mybir.AluOpType.add,
                        )
                    else:
                        d_chunk = d_sb[:, g * _GW : (g + 1) * _GW]
                        nc.vector.tensor_tensor(
                            out=d_chunk,
                            in0=ps[:],
                            in1=s_sb[:, g * _GW : (g + 1) * _GW],
                            op=mybir.AluOpType.subtract,
                        )
                        nc.vector.tensor_reduce(
                            gmaxs[:, g : g + 1],
                            d_chunk,
                            axis=mybir.AxisListType.X,
                            op=mybir.AluOpType.max,
                        )
                gmax = small.tile([128, 1], f32, tag="gmax")
                nc.vector.tensor_reduce(
                    gmax[:],
                    gmaxs[:],
                    axis=mybir.AxisListType.X,
                    op=mybir.AluOpType.max,
                )
                if _ARGMIN == "psum":
                    # winner = smallest global index among groups whose max
                    # equals the global max (matches jnp argmin tie-breaks)
                    gmaxb = small.tile([128, _GROUPS], f32, tag="gmaxb")
                    nc.vector.tensor_copy(
                        gmaxb[:], gmax[:, 0:1].to_broadcast([128, _GROUPS])
                    )
                    pred = small.tile([128, _GROUPS], u32, tag="pred")
                    nc.vector.tensor_tensor(
                        out=pred[:],
                        in0=gmaxs[:],
                        in1=gmaxb[:],
                        op=mybir.AluOpType.is_equal,
                    )
                    sel = small.tile([128, _GROUPS], u32, tag="sel")
                    nc.vector.memset(sel[:], 0xFFFFFFFF)
                    nc.vector.copy_predicated(sel[:], pred[:], gidx[:])
                    idxf = small.tile([128, 1], u32, tag="idxf")
                    nc.vector.tensor_reduce(
                        idxf[:],
                        sel[:],
                        axis=mybir.AxisListType.X,
                        op=mybir.AluOpType.min,
                    )
                    idx_ap = idxf[:, 0:1]
                else:
                    gmax8 = small.tile([128, 8], f32, tag="gmax8")
                    nc.vector.tensor_copy(
                        gmax8[:], gmax[:, 0:1].to_broadcast([128, 8])
                    )
                    idx8 = small.tile([128, 8], u32, tag="idx8")
                    nc.vector.max_index(idx8[:], gmax8[:], d_sb[:])
                    idx_ap = idx8[:, 0:1]
                quant_sb = qpool.tile([128, _D], f32, tag="q")
                nc.gpsimd.indirect_dma_start(
                    out=quant_sb[:],
                    out_offset=None,
                    in_=wrows,
                    in_offset=IndirectOffsetOnAxis(ap=idx_ap, axis=0),
                )
                sl = slice(m * 128, (m + 1) * 128)
                nc.sync.dma_start(quant_o[sl, :], quant_sb[:])
                nc.sync.dma_start(idx_o[sl, :], idx_ap)
                nc.sync.dma_start(gmin_o[sl, :], gmax[:])  # negated min dist
    return nc


def _split_multi_waits(nc):
    """The walrus build in this container rejects instructions carrying more
    than one sync wait ("Too many sync wait commands"). Hoist all but one
    wait of every instruction into standalone EventSemaphore instructions
    placed immediately before it on the same engine — semantically identical
    (all waits satisfied before the instruction issues)."""
    import bass_rust
    import concourse.mybir as mybir

    n_new = 0
    for fn in nc.m.functions:
        for bb in fn.blocks:
            insts = bb.instructions
            out = []
            changed = False
            for inst in insts:
                si = inst.sync_info
                if si is not None and si.on_wait and len(si.on_wait) > 1:
                    changed = True
                    waits = list(si.on_wait)
                    for w in waits[:-1]:
                        n_new += 1
                        out.append(
                            mybir.InstEventSemaphore(
                                name=f"I-wsplit-{n_new}",
                                engine=inst.engine,
                                ins=[],
                                outs=[],
                                sync_info=bass_rust.SyncInfo(
                                    on_wait=[w], on_update=[]
                                ),
                            )
                        )
                    inst.sync_info = bass_rust.SyncInfo(
                        on_wait=[waits[-1]], on_update=list(si.on_update or [])
                    )
                out.append(inst)
            if changed:
                bb.instructions = out
    return n_new


def _get_nc():
    key = (_PREC, _ARGMIN)
    if key not in _NC_CACHE:
        nc = _build()
        _split_multi_waits(nc)
        _NC_CACHE[key] = nc
    return _NC_CACHE[key]


def make_in_maps(hidden_state, emb_weight):
    hs = np.ascontiguousarray(np.asarray(hidden_state, dtype=np.float32))
    W = np.ascontiguousarray(np.asarray(emb_weight, dtype=np.float32))
    # ||e_k||^2 computed in f64 then rounded — the most faithful f32 value
    s64 = (W.astype(np.float64) ** 2).sum(1)
    if _ARGMIN == "psum":
        sneg = -s64
        r0 = sneg.astype(np.float16)
        r1 = (sneg - r0.astype(np.float64)).astype(np.float16)
        r2 = (sneg - r0.astype(np.float64) - r1.astype(np.float64)).astype(
            np.float16
        )
        sneg3 = np.ascontiguousarray(
            np.stack([r0, r1, r2, np.zeros(_K, np.float16)])
        )
    else:
        s32 = np.concatenate(
            [s64.astype(np.float32), [np.float32(np.finfo(np.float32).max)]]
        ).astype(np.float32)[None, :]
    wt = np.ascontiguousarray(W.T)  # [D, K]
    in_maps = []
    for c in range(_NCORES):
        hb = hs[2 * c : 2 * c + 2]  # [2, C, H, W]
        # x^T shard [D, 2048], token-major columns (b, h, w); 2x pre-scale is
        # exact (power of two) and folds the -2*x·e factor into the matmul.
        xt = (
            hb.reshape(2, _C, _H * _W).transpose(1, 0, 2).reshape(_C, _NSH)
        ) * np.float32(2.0)
        if _PREC == "fp32":
            xw0 = np.ascontiguousarray(
                np.concatenate([xt[0:128, :], wt[0:128, :]], axis=1)
            )
            xw1 = np.ascontiguousarray(
                np.concatenate([xt[128:256, :], wt[128:256, :]], axis=1)
            )
            m = {"xw0": xw0, "xw1": xw1}
        else:
            xth = xt.astype(np.float16)
            xtl = (xt - xth.astype(np.float32)).astype(np.float16)
            wth = wt.astype(np.float16)
            wtl = (wt - wth.astype(np.float32)).astype(np.float16)
            m = {
                "xh0": np.ascontiguousarray(xth[0:128]),
                "xl0": np.ascontiguousarray(xtl[0:128]),
                "xh1": np.ascontiguousarray(xth[128:256]),
                "xl1": np.ascontiguousarray(xtl[128:256]),
                "wh0": np.ascontiguousarray(wth[0:128]),
                "wl0": np.ascontiguousarray(wtl[0:128]),
                "wh1": np.ascontiguousarray(wth[128:256]),
                "wl1": np.ascontiguousarray(wtl[128:256]),
            }
        if _ARGMIN == "psum":
            m.update({"sneg3": sneg3, "wrows": W})
        else:
            m.update({"srep": s32, "wrows": W})
        in_maps.append(m)
    return hs, W, in_maps


def _ensure_ntff_hook():
    """bass_utils' axon trace path imports antenv.axon_hooks, which the
    agent image's antenv stub lacks. Register an equivalent module that
    drives NTFF profiling via ctypes against libaxon_pjrt.so."""
    import types

    if "antenv.axon_hooks" in sys.modules:
        return
    import contextlib
    import ctypes

    so_path = "/opt/axon/libaxon_pjrt.so"
    hook = None
    if os.path.exists(so_path):
        lib = ctypes.CDLL(so_path)
        if hasattr(lib, "axon_start_nrt_profile"):
            lib.axon_start_nrt_profile.argtypes = [
                ctypes.POINTER(ctypes.c_int64),
                ctypes.c_size_t,
            ]
            lib.axon_start_nrt_profile.restype = ctypes.c_int64
            lib.axon_stop_nrt_profile.argtypes = [ctypes.c_char_p]
            lib.axon_stop_nrt_profile.restype = ctypes.c_int64

            @contextlib.contextmanager
            def _hook(output_dir, device_ids):
                import jax

                jax.devices()
                if device_ids:
                    ids = (ctypes.c_int64 * len(device_ids))(*device_ids)
                    rc = lib.axon_start_nrt_profile(ids, len(device_ids))
                else:
                    rc = lib.axon_start_nrt_profile(None, 0)
                if rc != 0:
                    raise RuntimeError(f"axon_start_nrt_profile rc={rc}")
                try:
                    yield
                finally:
                    n = lib.axon_stop_nrt_profile(str(output_dir).encode())
                    print(f"profile: {n} ntff file(s) in {output_dir}", file=sys.stderr)

            hook = _hook

    mod = types.ModuleType("antenv.axon_hooks")
    mod.get_axon_ntff_profile_hook = lambda: hook
    mod.set_axon_ntff_profile_hook = lambda h: None
    sys.modules["antenv.axon_hooks"] = mod
    try:
        import antenv

        antenv.axon_hooks = mod
    except ImportError:
        pass


def _loss_like_reference(q_r, x_bhwc):
    """Replicate the reference's loss computation (two jnp.mean terms on the
    CPU backend) so the scalar matches its f32 accumulation order, not just
    the mathematical value."""
    try:
        import jax
        import jax.numpy as jnp

        cpu = jax.devices("cpu")[0]
        with jax.default_device(cpu):
            q = jnp.asarray(q_r)
            x = jnp.asarray(x_bhwc)
            loss = jnp.mean((q - x) ** 2) + np.float32(_BETA) * jnp.mean(
                (q - x) ** 2
            )
            return np.asarray(loss).astype(np.float32)
    except Exception:
        diff = q_r.astype(np.float64) - x_bhwc.astype(np.float64)
        return np.float32((1.0 + _BETA) * np.mean(diff**2))


def kernel(hidden_state, emb_weight):
    global LAST_RESULTS
    from concourse.bass_utils import run_bass_kernel_spmd

    hs, W, in_maps = make_in_maps(hidden_state, emb_weight)
    nc = _get_nc()
    trace = os.environ.get("VQ_TRACE", "0") == "1"
    if trace:
        _ensure_ntff_hook()
    res = run_bass_kernel_spmd(
        nc, in_maps, core_ids=list(range(_NCORES)), trace=trace
    )
    LAST_RESULTS = res
    idx = np.concatenate([r["idx"][:, 0] for r in res.results]).astype(np.int32)
    quant = np.concatenate([r["quant"] for r in res.results], axis=0)  # [N, D]

    x_bhwc = hs.transpose(0, 2, 3, 1)  # [B, H, W, C]
    q_r = quant.reshape(_B, _H, _W, _C)
    loss = _loss_like_reference(q_r, x_bhwc)
    quant_st = x_bhwc + (q_r - x_bhwc)  # straight-through forward
    quant_out = np.ascontiguousarray(quant_st.transpose(0, 3, 1, 2))
    min_idx = idx.reshape(_B, _H * _W)
    return quant_out, loss, min_idx
